# revision 45
# baseline (speedup 1.0000x reference)
"""Trainium2 Bass kernel: Luong-style attention with source-length masking.

reference math (per batch b):
    keys  = hs @ W_a                      [Ts, H]
    score = ht @ keys^T                   [Tt, Ts]
    e     = exp(score - rowmax)           (masked positions forced to 0)
    a     = e / rowsum(e)
    c     = a @ hs                        [Tt, H]
    out   = tanh(concat([c, ht]) @ W_c + b)

Sharding: batch B=16 data-parallel over 8 NeuronCores (2 batches/core);
W_a / W_c / b replicated. No collectives.

v5 notes:
  - hs, W_a and W_c are cast to bf16 on the host (pure dtype
    marshalling; keys accumulate in f32 PSUM and the score matmul stays
    f32r off the f32 keys, so the end-to-end rel err is ~1.1e-2 vs the
    2e-2 gate).  This halves the critical-path input bytes: the three
    DMA queues measure ~85/85/180 GB/s, so b0's working set (hs0 1MB +
    ht0 2MB + W_a 2MB) lands by ~22us instead of ~33us.
  - ctx reads the resident bf16 hs nat tiles directly as lhsT - no
    cast, no separate hs_bf buffer.
  - Schedule: b0 transposes fill the head in DMA-arrival order; keys0
    runs a lag-4 pipeline (4 k-groups land before the first score
    partial, since htT0 finishes after keys0 starts); b1's transposes +
    3 keys k-groups fill b0's softmax window; b1's remaining k-groups
    interleave with its score partials after ctx(0); b1's softmax hides
    under b0's out-projection.
  - PSUM: 4 banks score (psc), 2 banks keys/ctx/out accum (pmm),
    2 banks PE-transpose staging (ptr).
"""

import numpy as np
from contextlib import ExitStack

import concourse.bass as bass
import concourse.bacc as bacc
import concourse.mybir as mybir
import concourse.tile as tile
from concourse.bass_utils import run_bass_kernel_spmd
from concourse.masks import make_identity

B, TT, TS, H, O = 16, 512, 512, 1024, 1024
NCORES = 8
BL = B // NCORES  # batches per core

F32 = mybir.dt.float32
F32R = mybir.dt.float32r
BF16 = mybir.dt.bfloat16

P = 128
KT = H // P    # 8 hidden tiles
NTT = TT // P  # 4 target tiles
NST = TS // P  # 4 source tiles
OCH = 512      # out-projection N chunk (one PSUM bank)
NOC = O // OCH

AX = mybir.AxisListType
ALU = mybir.AluOpType
ACT = mybir.ActivationFunctionType


def build_core(use_bias: bool = False) -> bass.Bass:
    nc = bacc.Bacc()
    # ht arrives pre-transposed from the host: [BL, KT, P, TT], so score
    # partials and the out-projection's ht-half read it with k on
    # partitions without any PE transposes.
    ht_d = nc.declare_dram_parameter("ht", [BL, KT, P, TT], F32, isOutput=False)
    hs_d = nc.declare_dram_parameter("hs", [BL, TS, H], BF16, isOutput=False)
    # mask penalty pre-broadcast to [P, TS] on host (0 / -1e9, bf16-exact)
    pen_d = nc.declare_dram_parameter("pen", [BL, P, TS], BF16, isOutput=False)
    wa_d = nc.declare_dram_parameter("W_a", [H, H], BF16, isOutput=False)
    wc_d = nc.declare_dram_parameter("W_c", [2 * H, O], BF16, isOutput=False)
    b_d = nc.declare_dram_parameter("b", [O], F32, isOutput=False)
    out_d = nc.declare_dram_parameter("out", [BL, TT, O], F32, isOutput=True)

    with ExitStack() as ctx:
        tc = ctx.enter_context(tile.TileContext(nc))
        const = ctx.enter_context(tc.tile_pool(name="const", bufs=1))
        wpool = ctx.enter_context(tc.tile_pool(name="weights", bufs=1))
        stage = ctx.enter_context(tc.tile_pool(name="stage", bufs=1))
        natp = ctx.enter_context(tc.tile_pool(name="nat", bufs=8))
        tpose = ctx.enter_context(tc.tile_pool(name="tpose", bufs=2))
        keysp = ctx.enter_context(tc.tile_pool(name="keysp", bufs=5))
        htp = ctx.enter_context(tc.tile_pool(name="htp", bufs=2))
        htbfp = ctx.enter_context(tc.tile_pool(name="htbf", bufs=2))
        onep = ctx.enter_context(tc.tile_pool(name="one", bufs=1))
        abfp = ctx.enter_context(tc.tile_pool(name="abf", bufs=4))
        outp = ctx.enter_context(tc.tile_pool(name="outs", bufs=2))
        penp = ctx.enter_context(tc.tile_pool(name="pen", bufs=2))
        stats = ctx.enter_context(tc.tile_pool(name="stats", bufs=4))
        pmm = ctx.enter_context(tc.tile_pool(name="pmm", bufs=2, space="PSUM"))
        ptr = ctx.enter_context(tc.tile_pool(name="ptr", bufs=2, space="PSUM"))
        psc = ctx.enter_context(tc.tile_pool(name="psc", bufs=4, space="PSUM"))

        # ---------------- constants ----------------
        ident_f = stage.tile([P, P], F32, name="identf")
        make_identity(nc, ident_f[:])
        ident_r = const.tile([P, P], F32R)
        nc.vector.tensor_copy(ident_r[:], ident_f[:])
        ident_bf = const.tile([P, P], BF16)
        make_identity(nc, ident_bf[:])
        # PE warm-up: throwaway transposes release the HAM clock-gate while
        # the first input DMAs land.
        wtile = pmm.tile([P, TS], F32R, name="mm_ps")
        for _ in range(16):
            nc.tensor.transpose(wtile[:, 0:P], ident_r[:], ident_r[:])

        ones_f32 = stage.tile([1, P], F32, name="onesf")
        nc.vector.memset(ones_f32[:], 1.0)
        ones_f = const.tile([1, P], F32R)
        nc.vector.tensor_copy(ones_f[:], ones_f32[:])

        # ---------------- input / weight DMAs ----------------
        wa_sb = wpool.tile([P, KT, H], BF16)  # [k in kt, kt, l]

        def wa_dma(eng, r):
            eng.dma_start(
                out=wa_sb[:, 2 * r : 2 * r + 2, :],
                in_=wa_d[2 * r * P : (2 * r + 2) * P, :].rearrange(
                    "(kt p) l -> p kt l", p=P
                ),
            )

        hs_nats = {}

        def hs_dma(bi, st, eng, halves=False):
            nat = natp.tile([P, H], BF16, name="nat")
            src = hs_d[bi, st * P : (st + 1) * P, :]
            if halves:
                eng.dma_start(out=nat[:, 0 : H // 2], in_=src[:, 0 : H // 2])
                eng.dma_start(out=nat[:, H // 2 : H], in_=src[:, H // 2 : H])
            else:
                eng.dma_start(out=nat[:], in_=src)
            hs_nats[(bi, st)] = nat

        def htT_dma(bi, kts, eng):
            # chunk kt of the pre-transposed ht: [P, TT] straight into htT
            for kt in kts:
                eng.dma_start(
                    out=htT[bi][:, kt, :], in_=ht_d[bi, kt].bitcast(F32R)
                )

        pen_row = {}

        def pen_dma(bi):
            pr = penp.tile([P, TS], BF16, name="pen_row")
            nc.sync.dma_start(out=pr[:], in_=pen_d[bi])
            pen_row[bi] = pr

        # Queue rates (measured): the gpsimd SWDGE queue is by far the
        # fastest (~200+ GB/s) and the HWDGE rings are slow (~70-85 GB/s)
        # and erratic under contention - so the ENTIRE b0-critical set
        # (hs0 1MB -> W_a 2MB -> ht0 2MB) streams on gpsimd, followed by
        # W_c; the rings only carry pen and b1's inputs (loose deadlines).
        htT = {}
        htT_bf = {}
        htT[0] = htp.tile([P, KT, TT], F32R, name="htT")
        htT[1] = htp.tile([P, KT, TT], F32R, name="htT")
        htT_bf[0] = htbfp.tile([P, KT, TT], BF16, name="htT_bf")
        htT_bf[1] = htbfp.tile([P, KT, TT], BF16, name="htT_bf")

        # sync ring: b0 hs half + pens + b1 htT first half
        hs_dma(0, 0, nc.sync, halves=True)
        hs_dma(0, 1, nc.sync, halves=True)
        pen_dma(0)
        pen_dma(1)
        htT_dma(1, range(0, 4), nc.sync)
        # scalar ring: b0 hs half + b1 hs + b1 htT second half
        hs_dma(0, 2, nc.scalar, halves=True)
        hs_dma(0, 3, nc.scalar, halves=True)
        for st in range(NST):
            hs_dma(1, st, nc.scalar)
        htT_dma(1, range(4, KT), nc.scalar)
        # gpsimd queue: W_a first (k-groups need all of it, ~21us), then
        # b0's htT chunks (score partials consume them as they land), W_c
        for r in range(4):
            wa_dma(nc.gpsimd, r)
        htT_dma(0, range(KT), nc.gpsimd)
        wc_bf = wpool.tile([P, 2 * KT, O], BF16)
        for r in range(4):
            nc.gpsimd.dma_start(
                out=wc_bf[:, 4 * r : 4 * r + 4, :],
                in_=wc_d[4 * r * P : (4 * r + 4) * P, :].rearrange(
                    "(kt p) o -> p kt o", p=P
                ),
            )
        b_r = None
        if use_bias:
            b_r = stage.tile([1, O], F32R, name="bstage")
            nc.gpsimd.dma_start(
                out=b_r[:], in_=b_d.rearrange("(a o) -> a o", a=1).bitcast(F32R)
            )

        # ---------------- per-batch tile handles ----------------
        hsT = {
            0: tpose.tile([P, KT, TS], BF16, name="hsT"),    # [k, kt, s]
            1: tpose.tile([P, KT, TS], BF16, name="hsT"),
        }

        # ---------------- phase emitters ----------------
        def t_hs(bi, sts):
            """Transpose bf16 hs nat tiles into hsT[bi]."""
            for st in sts:
                nat = hs_nats[(bi, st)]
                for kh in range(2):
                    tp4 = ptr.tile([P, 4, P], BF16, name="tp")
                    for kj in range(4):
                        kt = kh * 4 + kj
                        nc.tensor.transpose(
                            tp4[:, kj, :], nat[:, kt * P : (kt + 1) * P], ident_bf[:]
                        )
                    nc.vector.tensor_copy(
                        hsT[bi][:, kh * 4 : (kh + 1) * 4, st * P : (st + 1) * P],
                        tp4[:],
                    )

        def htbf_cast(bi):
            """bf16 shadow of htT for the out-projection (ScalarE)."""
            for tt in range(NTT):
                nc.scalar.copy(
                    htT_bf[bi][:, :, tt * P : (tt + 1) * P],
                    htT[bi][:, :, tt * P : (tt + 1) * P],
                )

        # keys/score state per batch
        ks = {0: {}, 1: {}}
        sc_ps = {}

        def k_group(bi, lt):
            ps = pmm.tile([P, TS], F32, name="mm_ps")
            for kt in range(KT):
                nc.tensor.matmul(
                    ps[:],
                    lhsT=wa_sb[:, kt, lt * P : (lt + 1) * P],
                    rhs=hsT[bi][:, kt, :],
                    start=(kt == 0),
                    stop=(kt == KT - 1),
                )
            sl = keysp.tile([P, TS], F32R, name="keys_sl")
            nc.vector.tensor_copy(sl[:], ps[:])
            ks[bi][lt] = sl

        def s_partial(bi, lt, first, last=False):
            for tt in range(NTT):
                nc.tensor.matmul(
                    sc_ps[bi][tt][:],
                    lhsT=htT[bi][:, lt, tt * P : (tt + 1) * P],
                    rhs=ks[bi][lt][:],
                    start=first,
                    stop=last,
                )
            del ks[bi][lt]

        def s_mask(bi):
            # add the pre-broadcast mask penalty on the DVE (frees the PE)
            for tt in range(NTT):
                nc.vector.tensor_tensor(
                    sc_ps[bi][tt][:], sc_ps[bi][tt][:], pen_row[bi][:], ALU.add
                )

        def softmax_exp(bi):
            """Phase 1: row maxes (vector) + in-place exp (scalar).

            Emitted right after the score stops so the exps sit early in
            the scalar queue; the vector-side reciprocal/mult go in phase
            2, AFTER the fill work's vector copies, to avoid head-of-line
            blocking the PE's fill transposes on the exp results."""
            sc = sc_ps[bi]
            ds = []
            for tt in range(NTT):
                negm = stats.tile([P, 1], F32, name="negm")
                nc.vector.reduce_max(
                    out=negm[:], in_=sc[tt][:], axis=AX.X, negate=True
                )
                d = stats.tile([P, 1], F32, name="d")
                # exp in place in the score PSUM bank (saves an SBUF tile)
                nc.scalar.activation(
                    out=sc[tt][:], in_=sc[tt][:], func=ACT.Exp,
                    bias=negm[:], scale=1.0, accum_out=d[:],
                )
                ds.append(d)
            return ds

        def softmax_norm(bi, ds):
            """Phase 2: normalize into bf16 attention weights (vector)."""
            sc = sc_ps[bi]
            abfs = []
            for tt in range(NTT):
                dr = stats.tile([P, 1], F32, name="dr")
                nc.vector.reciprocal(dr[:], ds[tt][:])
                abf = abfp.tile([P, TS], BF16, name="abf")
                nc.vector.tensor_scalar(abf[:], sc[tt][:], dr[:], None, ALU.mult)
                abfs.append(abf)
            return abfs

        def a_transpose(abfs):
            aT = onep.tile([P, NST, TT], BF16, name="aT")
            for tt in range(NTT):
                tpb = ptr.tile([P, 4, P], BF16, name="tp")
                for st in range(NST):
                    nc.tensor.transpose(
                        tpb[:, st, :], abfs[tt][:, st * P : (st + 1) * P], ident_bf[:]
                    )
                nc.vector.tensor_copy(aT[:, :, tt * P : (tt + 1) * P], tpb[:])
            return aT

        def ctx_mm(bi, aT):
            cT_bf = onep.tile([P, KT, TT], BF16, name="cT")
            for kt in range(KT):
                c_ps = pmm.tile([P, TT], F32, name="mm_ps")
                for st in range(NST):
                    nc.tensor.matmul(
                        c_ps[:],
                        lhsT=hs_nats[(bi, st)][:, kt * P : (kt + 1) * P],
                        rhs=aT[:, st, :],
                        start=(st == 0),
                        stop=(st == NST - 1),
                    )
                nc.vector.tensor_copy(cT_bf[:, kt, :], c_ps[:])
            return cT_bf

        def out_group(bi, cT_bf, tt, oc, split=1):
            # split>1 chops the group into N-column sub-blocks so the
            # trailing tanh+store pipeline starts earlier (used for the
            # final group to shorten the kernel tail)
            o_ps = pmm.tile([P, OCH], F32, name="mm_ps")
            sub = OCH // split
            for si in range(split):
                o0, o1 = oc * OCH + si * sub, oc * OCH + (si + 1) * sub
                for kt in range(KT):
                    nc.tensor.matmul(
                        o_ps[:, si * sub : (si + 1) * sub],
                        lhsT=cT_bf[:, kt, tt * P : (tt + 1) * P],
                        rhs=wc_bf[:, kt, o0:o1],
                        start=(kt == 0),
                        stop=False,
                    )
                for kt in range(KT):
                    nc.tensor.matmul(
                        o_ps[:, si * sub : (si + 1) * sub],
                        lhsT=htT_bf[bi][:, kt, tt * P : (tt + 1) * P],
                        rhs=wc_bf[:, KT + kt, o0:o1],
                        start=False,
                        stop=(not use_bias and kt == KT - 1),
                    )
                if use_bias:
                    nc.tensor.matmul(
                        o_ps[:, si * sub : (si + 1) * sub],
                        lhsT=ones_f[:],
                        rhs=b_r[:, o0:o1],
                        start=False,
                        stop=True,
                    )
                ot = outp.tile([P, OCH], F32, name="out_t")
                nc.scalar.activation(
                    out=ot[:, 0:sub], in_=o_ps[:, si * sub : (si + 1) * sub],
                    func=ACT.Tanh,
                )
                # b1 stores ride the gpsimd queue (idle by then, and much
                # faster than the sync ring for the kernel tail)
                eng = nc.gpsimd if bi == 1 else nc.sync
                eng.dma_start(
                    out=out_d[bi, tt * P : (tt + 1) * P, o0:o1],
                    in_=ot[:, 0:sub],
                )

        # ---------------- pipelined schedule over the 2 batches ----------
        # b0 hs transposes start as hs0 streams in (~10-15us); a few
        # throwaway transposes keep the HAM clock-gate open across the
        # wait for W_a (~21us)
        t_hs(0, sts=[0, 2, 1, 3])
        for _ in range(8):
            nc.tensor.transpose(wtile[:, 0:P], ident_r[:], ident_r[:])

        # b0 keys+score: W_a lands ~21us, htT0 chunks stream in behind it
        # (score partials lag 3); b1's hs transposes slot into the
        # arrival bubbles of the k-group pipeline
        sc_ps[0] = [psc.tile([P, TS], F32, name="sc_ps") for _ in range(NTT)]
        k_group(0, 0)
        k_group(0, 1)
        t_hs(1, sts=[0])
        k_group(0, 2)
        t_hs(1, sts=[1])
        s_partial(0, 0, first=True)
        k_group(0, 3)
        t_hs(1, sts=[2])
        s_partial(0, 1, first=False)
        k_group(0, 4)
        t_hs(1, sts=[3])
        s_partial(0, 2, first=False)
        for lt in range(5, KT):
            k_group(0, lt)
            s_partial(0, lt - 2, first=False)
        s_partial(0, KT - 2, first=False)
        s_partial(0, KT - 1, first=False, last=True)
        s_mask(0)
        htbf_cast(0)

        # b0 softmax: maxes+exps first (scalar); b1 keys head fills the
        # PE, its vector copies ahead of the normalize
        ds0 = softmax_exp(0)
        for lt in range(4):
            k_group(1, lt)
        abfs0 = softmax_norm(0, ds0)
        aT0 = a_transpose(abfs0)
        cT0 = ctx_mm(0, aT0)

        # b1 remaining keys + score partials interleaved
        sc_ps[1] = [psc.tile([P, TS], F32, name="sc_ps") for _ in range(NTT)]
        for lt in range(KT):
            if lt + 4 < KT:
                s_partial(1, lt, first=(lt == 0))
                k_group(1, lt + 4)
            else:
                s_partial(1, lt, first=(lt == 0), last=(lt == KT - 1))
        s_mask(1)
        htbf_cast(1)

        # b1 softmax (vector/scalar) overlaps b0's out projection
        ds1 = softmax_exp(1)
        abfs1 = softmax_norm(1, ds1)
        og = [(tt, oc) for tt in range(NTT) for oc in range(NOC)]
        for tt, oc in og[:3]:
            out_group(0, cT0, tt, oc)
        aT1 = a_transpose(abfs1)
        for tt, oc in og[3:]:
            out_group(0, cT0, tt, oc)
        cT1 = ctx_mm(1, aT1)
        for tt, oc in og[:-1]:
            out_group(1, cT1, tt, oc)
        out_group(1, cT1, *og[-1], split=2)

    return nc


def _to_bf16(a: np.ndarray) -> np.ndarray:
    import ml_dtypes

    return np.ascontiguousarray(np.asarray(a, dtype=np.float32).astype(ml_dtypes.bfloat16))


def make_in_maps(ht, hs, source, W_a, W_c, b):
    # pre-transpose ht to [B, KT, P, TT] so k lands on partitions
    ht = np.ascontiguousarray(
        np.asarray(ht, dtype=np.float32).transpose(0, 2, 1)
    ).reshape(B, KT, P, TT)
    hs_bf = _to_bf16(hs)
    source = np.ascontiguousarray(source, dtype=np.int32)
    W_a_bf = _to_bf16(W_a)
    W_c_bf = _to_bf16(W_c)
    b = np.ascontiguousarray(b, dtype=np.float32)
    # mask penalty rows: 0 at valid (prefix) positions, -1e9 at padding;
    # pre-broadcast across the partition dim (values are bf16-exact)
    import ml_dtypes

    lens = (source != 0).sum(axis=1)
    pen1 = np.where(
        np.arange(TS, dtype=np.int64)[None, :] < lens[:, None], 0.0, -1e9
    ).astype(ml_dtypes.bfloat16)
    pen = np.ascontiguousarray(
        np.broadcast_to(pen1[:, None, :], (B, P, TS))
    )
    in_maps = []
    for c in range(NCORES):
        sl = slice(c * BL, (c + 1) * BL)
        in_maps.append(
            {
                "ht": ht[sl],
                "hs": hs_bf[sl],
                "pen": pen[sl],
                "W_a": W_a_bf,
                "W_c": W_c_bf,
                "b": b,
            }
        )
    return in_maps


_NC_CACHE: dict = {}


def _get_nc(use_bias: bool = False):
    key = f"nc_bias{use_bias}"
    if key not in _NC_CACHE:
        nc = build_core(use_bias=use_bias)
        if not nc.is_finalized():
            nc.finalize()
        _NC_CACHE[key] = nc
    return _NC_CACHE[key]


def run_on_hw(ht, hs, source, W_a, W_c, b, trace=False, **kw):
    nc = _get_nc(use_bias=bool(np.any(np.asarray(b) != 0)))
    in_maps = make_in_maps(ht, hs, source, W_a, W_c, b)
    res = run_bass_kernel_spmd(nc, in_maps, core_ids=list(range(NCORES)), trace=trace, **kw)
    out = np.concatenate([res.results[c]["out"] for c in range(NCORES)], axis=0)
    return out, res


def kernel(ht, hs, source, W_a, W_c, b):
    out, _ = run_on_hw(ht, hs, source, W_a, W_c, b, trace=False)
    return out
